# revision 13
# baseline (speedup 1.0000x reference)
"""Multi-head self-attention Trainium2 kernel (8 NeuronCores, SPMD).

Problem: x[B=4,N=2048,H=16,D=64], per-head Wq/Wk/Wv/Wo[H,D,D]+biases.
The computation is fully independent per (b,h) pair: 64 problems, 8/core.

Per-problem device layout (everything "transposed", i on free dim):
  xT_aug [65,2048]  = [x(b,:,h,:).T ; ones]          (host-prepped, bf16)
  qT = (Wq/32 | bq/32)-augmented proj, kT likewise (unscaled)
  v natural [2048,64] via 16 small matmuls -> v_aug [128, 16*(64+1)]
  sT[j,i] = sum_d kT[d,j] qT[d,i]
  p = exp(sT)  (no max subtraction: |s|<~1, softmax shift-invariant)
  attn_ext[f,i] = sum_j [v|1][j,f] p[j,i]  -> rows 0..63 attnT, row 64 sums
  out = (Wo^T @ attnT_raw + bo*sums) * (1/sums)  -> [64,2048] -> host .T

Performance structure (this environment: PE ~2.07GHz effective, ~1.5us
cross-engine semaphore latency):
 - scores contraction is K=64, so TWO j-tiles run CONCURRENTLY in the
   128x128 PE array via row tiling: qT is duplicated to partitions
   64..127 (qk2 [128,N]) and kT is laid out as j-tile pairs
   (kt2 [128, 8*128]: even j-tile on partitions 0..63, odd on 64..127).
   Each pair of row-packed matmuls fills one [128,1024] PSUM tile
   (= scores of 2 j-tiles for one i-quarter), exp'd by ONE FD=1024
   activation -> minimal ACT overhead (ACT is the roofline engine).
 - i is processed in quarters so the 2 live attention accumulators
   [65,512] cost 2 banks; 3 score slots cycle through the PE->ACT->PE
   ring to hide semaphore latency.
 - two problems are interleaved so both engines always have queued
   independent work; P@V matmuls lag exp by one step so the PE never
   stalls at its queue head.
"""

import numpy as np
import ml_dtypes

import concourse.bass as bass
import concourse.bacc as bacc
import concourse.mybir as mybir
from concourse.tile import TileContext
from concourse import bass_utils

B, N, H, D = 4, 2048, 16, 64
NCORES = 8
PPC = 8  # problems (b,h pairs) per core
DA = D + 1  # augmented (bias/ones) row count
JT = N // 128  # 16 j-tiles
JP = JT // 2  # 8 j-tile pairs
NQ = 512  # i-quarter width

F32 = mybir.dt.float32
BF16 = mybir.dt.bfloat16
EXP = mybir.ActivationFunctionType.Exp

_cache = {}


def _build(loop_n=1):
    if loop_n in _cache:
        return _cache[loop_n]
    nc = bacc.Bacc("TRN2", target_bir_lowering=False, debug=False, num_devices=NCORES)
    xt = nc.dram_tensor("xt", [PPC, DA, N], BF16, kind="ExternalInput")
    wt = nc.dram_tensor("wt", [DA, PPC * 4 * D], BF16, kind="ExternalInput")
    ot = nc.dram_tensor("ot", [PPC, D, N], F32, kind="ExternalOutput")

    with TileContext(nc) as tc:
        with (
            tc.tile_pool(name="w", bufs=1) as pw,
            tc.tile_pool(name="x", bufs=3) as px,
            tc.tile_pool(name="qk", bufs=6) as pqk,
            tc.tile_pool(name="v", bufs=3) as pv,
            tc.tile_pool(name="pt", bufs=6) as ppt,
            tc.tile_pool(name="misc", bufs=6) as pm,
            tc.tile_pool(name="out", bufs=3) as po,
            # 2-bank slots: [128,1024] f32 dual-jt scores (3 in ring) +
            # proj psum tiles
            tc.tile_pool(name="ps1", bufs=3, space="PSUM") as ps1,
            # 1-bank slots: [65,512] f32 attention accumulators (2 live)
            tc.tile_pool(name="ps_att", bufs=2, space="PSUM") as ps_att,
        ):
            w_all = pw.tile([DA, PPC * 4 * D], BF16, tag="w")
            nc.sync.dma_start(w_all[:], wt.ap())

            def proj(s):
                """Load x; qk2 [128,N] (qT duplicated), kt2 [128, 8*128]
                (j-tile pairs), v_aug [128, 16*65]."""
                woff = s * 4 * D
                xa = px.tile([DA, N], BF16, tag="x", name=f"xa{s}")
                nc.sync.dma_start(xa[:], xt.ap()[s])

                qk2 = pqk.tile([128, N], BF16, tag="qk", name=f"q2_{s}", bufs=3)
                kt2 = pqk.tile([128, N // 2], BF16, tag="kt", name=f"k2_{s}", bufs=3)
                for m in range(2):
                    for half in range(2):
                        ps = ps1.tile([D, N // 2], F32, tag="ps1", name="ps_p")
                        for c in range(2):
                            nc.tensor.matmul(
                                ps[:, c * NQ : (c + 1) * NQ],
                                w_all[:, woff + m * D : woff + (m + 1) * D],
                                xa[:, half * 1024 + c * NQ : half * 1024 + (c + 1) * NQ],
                                start=True,
                                stop=True,
                            )
                        if m == 0:
                            # qT -> both partition halves of qk2
                            sl = qk2[:, half * 1024 : (half + 1) * 1024]
                            nc.vector.tensor_copy(sl[0:D, :], ps[:])
                            nc.vector.tensor_copy(sl[D : D + 64, :], ps[:])
                        else:
                            # kT j-tiles: even -> rows 0:64, odd -> rows 64:128
                            # ps holds j-range half*1024..(half+1)*1024 = 8 jt
                            src = ps.rearrange("p (t w) -> p t w", w=128)
                            dst = kt2[:, half * 512 : (half + 1) * 512].rearrange(
                                "p (t w) -> p t w", w=128
                            )
                            nc.vector.tensor_copy(dst[0:D, :, :], src[:, 0::2, :])
                            nc.vector.tensor_copy(dst[D : D + 64, :, :], src[:, 1::2, :])

                v_ps = ps1.tile([128, JT * D], F32, tag="ps1", name="v_ps")
                for jt in range(JT):
                    nc.tensor.matmul(
                        v_ps[:, jt * D : (jt + 1) * D],
                        xa[:, jt * 128 : (jt + 1) * 128],
                        w_all[:, woff + 2 * D : woff + 3 * D],
                        start=True,
                        stop=True,
                    )
                v_aug = pv.tile([128, JT * (D + 1)], BF16, tag="v", name=f"v{s}")
                nc.gpsimd.memset(v_aug[:], 1.0)
                nc.vector.tensor_copy(
                    v_aug.rearrange("p (t c) -> p t c", c=D + 1)[:, :, 0:D],
                    v_ps.rearrange("p (t c) -> p t c", c=D),
                )
                return qk2, kt2, v_aug

            def tail(s, q, att_ps, o_sb):
                """out_quarter = (Wo^T @ attnT_raw + bo*sums) * (1/sums)."""
                woff = s * 4 * D
                a_bf = pm.tile([DA, NQ], BF16, tag="abf", name=f"abf{s}_{q}")
                nc.vector.tensor_copy(a_bf[:], att_ps[:])
                r = pm.tile([1, NQ], F32, tag="r", name=f"r{s}_{q}")
                nc.vector.reciprocal(r[:], att_ps[D : D + 1, :])
                r_b = pm.tile([D, NQ], F32, tag="rb", name=f"rb{s}_{q}")
                nc.gpsimd.partition_broadcast(r_b[:], r[:])
                ops = ps1.tile([D, NQ], F32, tag="ps1", name="ops")
                nc.tensor.matmul(
                    ops[:],
                    w_all[:, woff + 3 * D : woff + 4 * D],
                    a_bf[:],
                    start=True,
                    stop=True,
                )
                nc.vector.tensor_mul(o_sb[:, q * NQ : (q + 1) * NQ], ops[:], r_b[:])

            def pair(sa, sb):
                """Interleaved attention for problems sa, sb."""
                ctx = {}
                for s in (sa, sb):
                    qk2, kt2, v_aug = proj(s)
                    o_sb = po.tile([D, N], F32, tag="o", name=f"o{s}")
                    ctx[s] = (qk2, kt2, v_aug, o_sb)

                # Flat software pipeline over (q, p) steps: att matmuls lag
                # sc/exp by LAG steps (across quarter boundaries), and each
                # quarter's tails are emitted inside the next quarter's
                # stream so the PE never stalls on the DVE normalize chain.
                LAG = 3
                att = {}
                pts = {}

                def sc_exp(s, q, p):
                    qk2, kt2, _, _ = ctx[s]
                    sp = ps1.tile([128, 2 * NQ], F32, tag="ps1", name="sps")
                    for par in range(2):  # even/odd j-tile, row-packed
                        nc.tensor.matmul(
                            sp[:, par * NQ : (par + 1) * NQ],
                            kt2[par * D : par * D + D, p * 128 : (p + 1) * 128],
                            qk2[par * D : par * D + D, q * NQ : (q + 1) * NQ],
                            start=True,
                            stop=True,
                        )
                    pt = ppt.tile([128, 2 * NQ], BF16, tag="pt", name="pt")
                    nc.scalar.activation(pt[:], sp[:], EXP)
                    pts[(s, q, p)] = pt

                def att_mm(s, q, p):
                    _, _, v_aug, _ = ctx[s]
                    if p == 0:
                        att[(s, q)] = ps_att.tile(
                            [DA, NQ], F32, tag="att", name=f"att{s}_{q}"
                        )
                    pt = pts.pop((s, q, p))
                    for par in range(2):
                        jt = 2 * p + par
                        nc.tensor.matmul(
                            att[(s, q)][:],
                            v_aug[:, jt * (D + 1) : (jt + 1) * (D + 1)],
                            pt[:, par * NQ : (par + 1) * NQ],
                            start=(jt == 0),
                            stop=(jt == JT - 1),
                        )

                NSTEP = 4 * JP
                for g in range(NSTEP + LAG + 3):
                    if g < NSTEP:
                        q, p = divmod(g, JP)
                        sc_exp(sa, q, p)
                        sc_exp(sb, q, p)
                    if LAG <= g < NSTEP + LAG:
                        q, p = divmod(g - LAG, JP)
                        att_mm(sa, q, p)
                        att_mm(sb, q, p)
                    # tails for quarter q' trail its last att step by 3
                    gt = g - LAG - 3
                    if gt >= 0 and gt % JP == JP - 1:
                        qt = gt // JP
                        for s in (sa, sb):
                            tail(s, qt, att.pop((s, qt)), ctx[s][3])

                for s in (sa, sb):
                    nc.sync.dma_start(ot.ap()[s], ctx[s][3][:])

            def body():
                for sp in range(PPC // 2):
                    pair(2 * sp, 2 * sp + 1)

            if loop_n > 1:
                with tc.For_i(0, loop_n, 1):
                    body()
            else:
                body()

    nc.compile()
    _cache[loop_n] = nc
    return nc


def _host_prep(x, Wq, bq, Wk, bk, Wv, bv, Wo, bo):
    """Returns per-core in_maps."""
    x = np.asarray(x, np.float32)
    scale = 1.0 / np.sqrt(np.float32(H * D))
    in_maps = []
    for c in range(NCORES):
        xt = np.empty((PPC, DA, N), ml_dtypes.bfloat16)
        wt = np.empty((DA, PPC * 4 * D), np.float32)
        for s in range(PPC):
            p = c * PPC + s
            b, h = divmod(p, H)
            xt[s, :D, :] = x[b, :, h, :].T.astype(ml_dtypes.bfloat16)
            xt[s, D, :] = 1.0
            o = s * 4 * D
            wt[:D, o : o + D] = Wq[h] * scale
            wt[D, o : o + D] = bq[h] * scale
            wt[:D, o + D : o + 2 * D] = Wk[h]
            wt[D, o + D : o + 2 * D] = bk[h]
            wt[:D, o + 2 * D : o + 3 * D] = Wv[h]
            wt[D, o + 2 * D : o + 3 * D] = bv[h]
            wt[:D, o + 3 * D : o + 4 * D] = Wo[h]
            wt[D, o + 3 * D : o + 4 * D] = bo[h]
        in_maps.append({"xt": xt, "wt": wt.astype(ml_dtypes.bfloat16)})
    return in_maps


def _gather(results):
    out = np.empty((B, N, H, D), np.float32)
    for c in range(NCORES):
        ot = results[c]["ot"]
        for s in range(PPC):
            b, h = divmod(c * PPC + s, H)
            out[b, :, h, :] = ot[s].T
    return out


def run(in_maps, loop_n=1, **kw):
    nc = _build(loop_n)
    return bass_utils.run_bass_kernel_spmd(
        nc, in_maps, core_ids=list(range(NCORES)), **kw
    )


def kernel(x, Wq, bq, Wk, bk, Wv, bv, Wo, bo):
    in_maps = _host_prep(x, Wq, bq, Wk, bk, Wv, bv, Wo, bo)
    res = run(in_maps)
    return _gather(res.results)


# revision 14
# speedup vs baseline: 1.2866x; 1.2866x over previous
"""Multi-head self-attention Trainium2 kernel (8 NeuronCores, SPMD).

Problem: x[B=4,N=2048,H=16,D=64], per-head Wq/Wk/Wv/Wo[H,D,D]+biases.
The computation is fully independent per (b,h) pair: 64 problems, 8/core.

Per-problem device layout (everything "transposed", i on free dim):
  xT_aug [65,2048]  = [x(b,:,h,:).T ; ones]          (host-prepped, bf16)
  qT = (Wq/32 | bq/32)-augmented proj, kT likewise (unscaled)
  v natural [2048,64] via 16 small matmuls -> v_aug [128, 16*(64+1)]
  sT[j,i] = sum_d kT[d,j] qT[d,i]
  p = exp(sT)  (no max subtraction: |s|<~1, softmax shift-invariant)
  attn_ext[f,i] = sum_j [v|1][j,f] p[j,i]  -> rows 0..63 attnT, row 64 sums
  out = (Wo^T @ attnT_raw + bo*sums) * (1/sums)  -> [64,2048] -> host .T

Performance structure (this environment: PE ~2.07GHz effective, ~1.5us
cross-engine semaphore latency):
 - scores contraction is K=64, so TWO j-tiles run CONCURRENTLY in the
   128x128 PE array via row tiling: qT is duplicated to partitions
   64..127 (qk2 [128,N]) and kT is laid out as j-tile pairs
   (kt2 [128, 8*128]: even j-tile on partitions 0..63, odd on 64..127).
   Each pair of row-packed matmuls fills one [128,1024] PSUM tile
   (= scores of 2 j-tiles for one i-quarter), exp'd by ONE FD=1024
   activation -> minimal ACT overhead (ACT is the roofline engine).
 - i is processed in quarters so the 2 live attention accumulators
   [65,512] cost 2 banks; 3 score slots cycle through the PE->ACT->PE
   ring to hide semaphore latency.
 - two problems are interleaved so both engines always have queued
   independent work; P@V matmuls lag exp by one step so the PE never
   stalls at its queue head.
"""

import numpy as np
import ml_dtypes

import concourse.bass as bass
import concourse.bacc as bacc
import concourse.mybir as mybir
from concourse.tile import TileContext
from concourse import bass_utils

B, N, H, D = 4, 2048, 16, 64
NCORES = 8
PPC = 8  # problems (b,h pairs) per core
DA = D + 1  # augmented (bias/ones) row count
JT = N // 128  # 16 j-tiles
JP = JT // 2  # 8 j-tile pairs
NQ = 512  # i-quarter width

F32 = mybir.dt.float32
BF16 = mybir.dt.bfloat16
EXP = mybir.ActivationFunctionType.Exp

_cache = {}


def _build(loop_n=1):
    if loop_n in _cache:
        return _cache[loop_n]
    nc = bacc.Bacc("TRN2", target_bir_lowering=False, debug=False, num_devices=NCORES)
    xt = nc.dram_tensor("xt", [PPC, DA, N], BF16, kind="ExternalInput")
    wt = nc.dram_tensor("wt", [DA, PPC * 4 * D], BF16, kind="ExternalInput")
    ot = nc.dram_tensor("ot", [PPC, D, N], F32, kind="ExternalOutput")

    with TileContext(nc) as tc:
        with (
            tc.tile_pool(name="w", bufs=1) as pw,
            tc.tile_pool(name="x", bufs=3) as px,
            tc.tile_pool(name="qk", bufs=6) as pqk,
            tc.tile_pool(name="v", bufs=3) as pv,
            tc.tile_pool(name="pt", bufs=12) as ppt,
            tc.tile_pool(name="misc", bufs=6) as pm,
            tc.tile_pool(name="out", bufs=3) as po,
            # 2-bank slots: [128,1024] f32 dual-jt scores (3 in ring) +
            # proj psum tiles
            tc.tile_pool(name="ps1", bufs=3, space="PSUM") as ps1,
            # 1-bank slots: [65,512] f32 attention accumulators (2 live)
            tc.tile_pool(name="ps_att", bufs=2, space="PSUM") as ps_att,
        ):
            w_all = pw.tile([DA, PPC * 4 * D], BF16, tag="w")
            nc.sync.dma_start(w_all[:], wt.ap())

            def proj(s):
                """Load x; qk2 [128,N] (qT duplicated), kt2 [128, 8*128]
                (j-tile pairs), v_aug [128, 16*65]."""
                woff = s * 4 * D
                xa = px.tile([DA, N], BF16, tag="x", name=f"xa{s}")
                nc.sync.dma_start(xa[:], xt.ap()[s])

                qk2 = pqk.tile([128, N], BF16, tag="qk", name=f"q2_{s}", bufs=3)
                kt2 = pqk.tile([128, N // 2], BF16, tag="kt", name=f"k2_{s}", bufs=3)
                for m in range(2):
                    for half in range(2):
                        ps = ps1.tile([D, N // 2], F32, tag="ps1", name="ps_p")
                        for c in range(2):
                            nc.tensor.matmul(
                                ps[:, c * NQ : (c + 1) * NQ],
                                w_all[:, woff + m * D : woff + (m + 1) * D],
                                xa[:, half * 1024 + c * NQ : half * 1024 + (c + 1) * NQ],
                                start=True,
                                stop=True,
                            )
                        if m == 0:
                            # qT -> both partition halves of qk2
                            sl = qk2[:, half * 1024 : (half + 1) * 1024]
                            nc.vector.tensor_copy(sl[0:D, :], ps[:])
                            nc.vector.tensor_copy(sl[D : D + 64, :], ps[:])
                        else:
                            # kT j-tiles: even -> rows 0:64, odd -> rows 64:128
                            # ps holds j-range half*1024..(half+1)*1024 = 8 jt
                            src = ps.rearrange("p (t w) -> p t w", w=128)
                            dst = kt2[:, half * 512 : (half + 1) * 512].rearrange(
                                "p (t w) -> p t w", w=128
                            )
                            nc.vector.tensor_copy(dst[0:D, :, :], src[:, 0::2, :])
                            nc.vector.tensor_copy(dst[D : D + 64, :, :], src[:, 1::2, :])

                v_ps = ps1.tile([128, JT * D], F32, tag="ps1", name="v_ps")
                for jt in range(JT):
                    nc.tensor.matmul(
                        v_ps[:, jt * D : (jt + 1) * D],
                        xa[:, jt * 128 : (jt + 1) * 128],
                        w_all[:, woff + 2 * D : woff + 3 * D],
                        start=True,
                        stop=True,
                    )
                v_aug = pv.tile([128, JT * (D + 1)], BF16, tag="v", name=f"v{s}")
                nc.gpsimd.memset(v_aug[:], 1.0)
                nc.vector.tensor_copy(
                    v_aug.rearrange("p (t c) -> p t c", c=D + 1)[:, :, 0:D],
                    v_ps.rearrange("p (t c) -> p t c", c=D),
                )
                return qk2, kt2, v_aug

            def tail(s, q, att_ps, o_sb):
                """out_quarter = (Wo^T @ attnT_raw + bo*sums) * (1/sums)."""
                woff = s * 4 * D
                a_bf = pm.tile([DA, NQ], BF16, tag="abf", name=f"abf{s}_{q}")
                nc.vector.tensor_copy(a_bf[:], att_ps[:])
                r = pm.tile([1, NQ], F32, tag="r", name=f"r{s}_{q}")
                nc.vector.reciprocal(r[:], att_ps[D : D + 1, :])
                r_b = pm.tile([D, NQ], F32, tag="rb", name=f"rb{s}_{q}")
                nc.gpsimd.partition_broadcast(r_b[:], r[:])
                ops = ps1.tile([D, NQ], F32, tag="ps1", name="ops")
                nc.tensor.matmul(
                    ops[:],
                    w_all[:, woff + 3 * D : woff + 4 * D],
                    a_bf[:],
                    start=True,
                    stop=True,
                )
                nc.vector.tensor_mul(o_sb[:, q * NQ : (q + 1) * NQ], ops[:], r_b[:])

            def pair(sa, sb):
                """Interleaved attention for problems sa, sb."""
                ctx = {}
                for s in (sa, sb):
                    qk2, kt2, v_aug = proj(s)
                    o_sb = po.tile([D, N], F32, tag="o", name=f"o{s}")
                    ctx[s] = (qk2, kt2, v_aug, o_sb)

                # Flat software pipeline over (q, p) steps: att matmuls lag
                # sc/exp by LAG steps (across quarter boundaries), and each
                # quarter's tails are emitted inside the next quarter's
                # stream so the PE never stalls on the DVE normalize chain.
                LAG = 3
                att = {}
                pts = {}

                def sc_exp(s, q, p):
                    qk2, kt2, _, _ = ctx[s]
                    sp = ps1.tile([128, 2 * NQ], F32, tag="ps1", name="sps")
                    for par in range(2):  # even/odd j-tile, row-packed
                        nc.tensor.matmul(
                            sp[:, par * NQ : (par + 1) * NQ],
                            kt2[par * D : par * D + D, p * 128 : (p + 1) * 128],
                            qk2[par * D : par * D + D, q * NQ : (q + 1) * NQ],
                            start=True,
                            stop=True,
                        )
                    pt = ppt.tile([128, 2 * NQ], BF16, tag="pt", name="pt")
                    nc.scalar.activation(pt[:], sp[:], EXP)
                    pts[(s, q, p)] = pt

                def att_mm(s, q, p):
                    _, _, v_aug, _ = ctx[s]
                    if p == 0:
                        att[(s, q)] = ps_att.tile(
                            [DA, NQ], F32, tag="att", name=f"att{s}_{q}"
                        )
                    pt = pts.pop((s, q, p))
                    for par in range(2):
                        jt = 2 * p + par
                        nc.tensor.matmul(
                            att[(s, q)][:],
                            v_aug[:, jt * (D + 1) : (jt + 1) * (D + 1)],
                            pt[:, par * NQ : (par + 1) * NQ],
                            start=(jt == 0),
                            stop=(jt == JT - 1),
                        )

                NSTEP = 4 * JP
                for g in range(NSTEP + LAG + 3):
                    if g < NSTEP:
                        q, p = divmod(g, JP)
                        sc_exp(sa, q, p)
                        sc_exp(sb, q, p)
                    if LAG <= g < NSTEP + LAG:
                        q, p = divmod(g - LAG, JP)
                        att_mm(sa, q, p)
                        att_mm(sb, q, p)
                    # tails for quarter q' trail its last att step by 3
                    gt = g - LAG - 3
                    if gt >= 0 and gt % JP == JP - 1:
                        qt = gt // JP
                        for s in (sa, sb):
                            tail(s, qt, att.pop((s, qt)), ctx[s][3])

                for s in (sa, sb):
                    nc.sync.dma_start(ot.ap()[s], ctx[s][3][:])

            def body():
                for sp in range(PPC // 2):
                    pair(2 * sp, 2 * sp + 1)

            if loop_n > 1:
                with tc.For_i(0, loop_n, 1):
                    body()
            else:
                body()

    nc.compile()
    _cache[loop_n] = nc
    return nc


def _host_prep(x, Wq, bq, Wk, bk, Wv, bv, Wo, bo):
    """Returns per-core in_maps."""
    x = np.asarray(x, np.float32)
    scale = 1.0 / np.sqrt(np.float32(H * D))
    in_maps = []
    for c in range(NCORES):
        xt = np.empty((PPC, DA, N), ml_dtypes.bfloat16)
        wt = np.empty((DA, PPC * 4 * D), np.float32)
        for s in range(PPC):
            p = c * PPC + s
            b, h = divmod(p, H)
            xt[s, :D, :] = x[b, :, h, :].T.astype(ml_dtypes.bfloat16)
            xt[s, D, :] = 1.0
            o = s * 4 * D
            wt[:D, o : o + D] = Wq[h] * scale
            wt[D, o : o + D] = bq[h] * scale
            wt[:D, o + D : o + 2 * D] = Wk[h]
            wt[D, o + D : o + 2 * D] = bk[h]
            wt[:D, o + 2 * D : o + 3 * D] = Wv[h]
            wt[D, o + 2 * D : o + 3 * D] = bv[h]
            wt[:D, o + 3 * D : o + 4 * D] = Wo[h]
            wt[D, o + 3 * D : o + 4 * D] = bo[h]
        in_maps.append({"xt": xt, "wt": wt.astype(ml_dtypes.bfloat16)})
    return in_maps


def _gather(results):
    out = np.empty((B, N, H, D), np.float32)
    for c in range(NCORES):
        ot = results[c]["ot"]
        for s in range(PPC):
            b, h = divmod(c * PPC + s, H)
            out[b, :, h, :] = ot[s].T
    return out


def run(in_maps, loop_n=1, **kw):
    nc = _build(loop_n)
    return bass_utils.run_bass_kernel_spmd(
        nc, in_maps, core_ids=list(range(NCORES)), **kw
    )


def kernel(x, Wq, bq, Wk, bk, Wv, bv, Wo, bo):
    in_maps = _host_prep(x, Wq, bq, Wk, bk, Wv, bv, Wo, bo)
    res = run(in_maps)
    return _gather(res.results)


# revision 15
# speedup vs baseline: 1.4839x; 1.1533x over previous
"""Multi-head self-attention Trainium2 kernel (8 NeuronCores, SPMD).

Problem: x[B=4,N=2048,H=16,D=64], per-head Wq/Wk/Wv/Wo[H,D,D]+biases.
The computation is fully independent per (b,h) pair: 64 problems, 8/core.

Per-problem device layout (everything "transposed", i on free dim):
  xT_aug [65,2048]  = [x(b,:,h,:).T ; ones]          (host-prepped, bf16)
  qT = (Wq/32 | bq/32)-augmented proj, kT likewise (unscaled)
  v natural [2048,64] via 16 small matmuls -> v_aug [128, 16*(64+1)]
  sT[j,i] = sum_d kT[d,j] qT[d,i]
  p = exp(sT)  (no max subtraction: |s|<~1, softmax shift-invariant)
  attn_ext[f,i] = sum_j [v|1][j,f] p[j,i]  -> rows 0..63 attnT, row 64 sums
  out = (Wo^T @ attnT_raw + bo*sums) * (1/sums)  -> [64,2048] -> host .T

Performance structure (this environment: PE ~2.07GHz effective, ~1.5us
cross-engine semaphore latency):
 - scores contraction is K=64, so TWO j-tiles run CONCURRENTLY in the
   128x128 PE array via row tiling: qT is duplicated to partitions
   64..127 (qk2 [128,N]) and kT is laid out as j-tile pairs
   (kt2 [128, 8*128]: even j-tile on partitions 0..63, odd on 64..127).
   Each pair of row-packed matmuls fills one [128,1024] PSUM tile
   (= scores of 2 j-tiles for one i-quarter), exp'd by ONE FD=1024
   activation -> minimal ACT overhead (ACT is the roofline engine).
 - i is processed in quarters so the 2 live attention accumulators
   [65,512] cost 2 banks; 3 score slots cycle through the PE->ACT->PE
   ring to hide semaphore latency.
 - two problems are interleaved so both engines always have queued
   independent work; P@V matmuls lag exp by one step so the PE never
   stalls at its queue head.
"""

import numpy as np
import ml_dtypes

import concourse.bass as bass
import concourse.bacc as bacc
import concourse.mybir as mybir
from concourse.tile import TileContext
from concourse import bass_utils

B, N, H, D = 4, 2048, 16, 64
NCORES = 8
PPC = 8  # problems (b,h pairs) per core
DA = D + 1  # augmented (bias/ones) row count
JT = N // 128  # 16 j-tiles
JP = JT // 2  # 8 j-tile pairs
NQ = 512  # i-quarter width

F32 = mybir.dt.float32
BF16 = mybir.dt.bfloat16
EXP = mybir.ActivationFunctionType.Exp

_cache = {}


def _build(loop_n=1):
    if loop_n in _cache:
        return _cache[loop_n]
    nc = bacc.Bacc("TRN2", target_bir_lowering=False, debug=False, num_devices=NCORES)
    xt = nc.dram_tensor("xt", [PPC, DA, N], BF16, kind="ExternalInput")
    wt = nc.dram_tensor("wt", [DA, PPC * 4 * D], BF16, kind="ExternalInput")
    ot = nc.dram_tensor("ot", [PPC, D, N], F32, kind="ExternalOutput")

    with TileContext(nc) as tc:
        with (
            tc.tile_pool(name="w", bufs=1) as pw,
            tc.tile_pool(name="x", bufs=3) as px,
            tc.tile_pool(name="qk", bufs=6) as pqk,
            tc.tile_pool(name="v", bufs=3) as pv,
            tc.tile_pool(name="pt", bufs=12) as ppt,
            tc.tile_pool(name="misc", bufs=6) as pm,
            tc.tile_pool(name="out", bufs=3) as po,
            # 2-bank slots: [128,1024] f32 dual-jt scores (3 in ring) +
            # proj psum tiles
            tc.tile_pool(name="ps1", bufs=3, space="PSUM") as ps1,
            # 1-bank slots: [65,512] f32 attention accumulators (2 live)
            tc.tile_pool(name="ps_att", bufs=2, space="PSUM") as ps_att,
        ):
            w_all = pw.tile([DA, PPC * 4 * D], BF16, tag="w")
            nc.sync.dma_start(w_all[:], wt.ap())

            def proj(s):
                """Load x; qk2 [128,N] (qT duplicated), kt2 [128, 8*128]
                (j-tile pairs), v_aug [128, 16*65]."""
                woff = s * 4 * D
                xa = px.tile([DA, N], BF16, tag="x", name=f"xa{s}")
                nc.sync.dma_start(xa[:], xt.ap()[s])

                qk2 = pqk.tile([128, N], BF16, tag="qk", name=f"q2_{s}", bufs=3)
                kt2 = pqk.tile([128, N // 2], BF16, tag="kt", name=f"k2_{s}", bufs=3)
                for m in range(2):
                    for half in range(2):
                        ps = ps1.tile([D, N // 2], F32, tag="ps1", name="ps_p")
                        for c in range(2):
                            nc.tensor.matmul(
                                ps[:, c * NQ : (c + 1) * NQ],
                                w_all[:, woff + m * D : woff + (m + 1) * D],
                                xa[:, half * 1024 + c * NQ : half * 1024 + (c + 1) * NQ],
                                start=True,
                                stop=True,
                            )
                        if m == 0:
                            # qT -> both partition halves of qk2
                            sl = qk2[:, half * 1024 : (half + 1) * 1024]
                            nc.vector.tensor_copy(sl[0:D, :], ps[:])
                            nc.vector.tensor_copy(sl[D : D + 64, :], ps[:])
                        else:
                            # kT j-tiles: even -> rows 0:64, odd -> rows 64:128
                            # ps holds j-range half*1024..(half+1)*1024 = 8 jt
                            src = ps.rearrange("p (t w) -> p t w", w=128)
                            dst = kt2[:, half * 512 : (half + 1) * 512].rearrange(
                                "p (t w) -> p t w", w=128
                            )
                            nc.vector.tensor_copy(dst[0:D, :, :], src[:, 0::2, :])
                            nc.vector.tensor_copy(dst[D : D + 64, :, :], src[:, 1::2, :])

                v_ps = ps1.tile([128, JT * D], F32, tag="ps1", name="v_ps")
                for jt in range(JT):
                    nc.tensor.matmul(
                        v_ps[:, jt * D : (jt + 1) * D],
                        xa[:, jt * 128 : (jt + 1) * 128],
                        w_all[:, woff + 2 * D : woff + 3 * D],
                        start=True,
                        stop=True,
                    )
                v_aug = pv.tile([128, JT * (D + 1)], BF16, tag="v", name=f"v{s}")
                nc.gpsimd.memset(v_aug[:], 1.0)
                nc.vector.tensor_copy(
                    v_aug.rearrange("p (t c) -> p t c", c=D + 1)[:, :, 0:D],
                    v_ps.rearrange("p (t c) -> p t c", c=D),
                )
                return qk2, kt2, v_aug

            def tail(s, q, att_ps, o_sb):
                """out_quarter = (Wo^T @ attnT_raw + bo*sums) * (1/sums)."""
                woff = s * 4 * D
                a_bf = pm.tile([DA, NQ], BF16, tag="abf", name=f"abf{s}_{q}")
                nc.vector.tensor_copy(a_bf[:], att_ps[:])
                r = pm.tile([1, NQ], F32, tag="r", name=f"r{s}_{q}")
                nc.vector.reciprocal(r[:], att_ps[D : D + 1, :])
                r_b = pm.tile([D, NQ], F32, tag="rb", name=f"rb{s}_{q}")
                nc.gpsimd.partition_broadcast(r_b[:], r[:])
                ops = ps1.tile([D, NQ], F32, tag="ps1", name="ops")
                nc.tensor.matmul(
                    ops[:],
                    w_all[:, woff + 3 * D : woff + 4 * D],
                    a_bf[:],
                    start=True,
                    stop=True,
                )
                nc.vector.tensor_mul(o_sb[:, q * NQ : (q + 1) * NQ], ops[:], r_b[:])

            def pair(sa, sb):
                """Interleaved attention for problems sa, sb."""
                ctx = {}
                for s in (sa, sb):
                    qk2, kt2, v_aug = proj(s)
                    o_sb = po.tile([D, N], F32, tag="o", name=f"o{s}")
                    ctx[s] = (qk2, kt2, v_aug, o_sb)

                # Flat software pipeline over (q, p) steps: att matmuls lag
                # sc/exp by LAG steps (across quarter boundaries), and each
                # quarter's tails are emitted inside the next quarter's
                # stream so the PE never stalls on the DVE normalize chain.
                LAG = 3
                att = {}
                pts = {}

                def sc_exp(s, q, p):
                    qk2, kt2, _, _ = ctx[s]
                    sp = ps1.tile([128, 2 * NQ], F32, tag="ps1", name="sps")
                    for par in range(2):  # even/odd j-tile, row-packed
                        nc.tensor.matmul(
                            sp[:, par * NQ : (par + 1) * NQ],
                            kt2[par * D : par * D + D, p * 128 : (p + 1) * 128],
                            qk2[par * D : par * D + D, q * NQ : (q + 1) * NQ],
                            start=True,
                            stop=True,
                        )
                    pt = ppt.tile([128, 2 * NQ], BF16, tag="pt", name="pt")
                    nc.scalar.activation(pt[:], sp[:], EXP)
                    pts[(s, q, p)] = pt

                def att_mm(s, q, p):
                    _, _, v_aug, _ = ctx[s]
                    if p == 0:
                        att[(s, q)] = ps_att.tile(
                            [DA, NQ], F32, tag="att", name=f"att{s}_{q}"
                        )
                    pt = pts.pop((s, q, p))
                    for par in range(2):
                        jt = 2 * p + par
                        nc.tensor.matmul(
                            att[(s, q)][:],
                            v_aug[:, jt * (D + 1) : (jt + 1) * (D + 1)],
                            pt[:, par * NQ : (par + 1) * NQ],
                            start=(jt == 0),
                            stop=(jt == JT - 1),
                        )

                NSTEP = 4 * JP
                for g in range(NSTEP + LAG + 3):
                    if g < NSTEP:
                        q, p = divmod(g, JP)
                        sc_exp(sa, q, p)
                        sc_exp(sb, q, p)
                    if LAG <= g < NSTEP + LAG:
                        q, p = divmod(g - LAG, JP)
                        att_mm(sa, q, p)
                        att_mm(sb, q, p)
                    # tails for quarter q' trail its last att step by 3
                    gt = g - LAG
                    if gt >= 0 and gt % JP == JP - 1:
                        qt = gt // JP
                        for s in (sa, sb):
                            tail(s, qt, att.pop((s, qt)), ctx[s][3])

                for s in (sa, sb):
                    nc.sync.dma_start(ot.ap()[s], ctx[s][3][:])

            def body():
                for sp in range(PPC // 2):
                    pair(2 * sp, 2 * sp + 1)

            if loop_n > 1:
                with tc.For_i(0, loop_n, 1):
                    body()
            else:
                body()

    nc.compile()
    _cache[loop_n] = nc
    return nc


def _host_prep(x, Wq, bq, Wk, bk, Wv, bv, Wo, bo):
    """Returns per-core in_maps."""
    x = np.asarray(x, np.float32)
    scale = 1.0 / np.sqrt(np.float32(H * D))
    in_maps = []
    for c in range(NCORES):
        xt = np.empty((PPC, DA, N), ml_dtypes.bfloat16)
        wt = np.empty((DA, PPC * 4 * D), np.float32)
        for s in range(PPC):
            p = c * PPC + s
            b, h = divmod(p, H)
            xt[s, :D, :] = x[b, :, h, :].T.astype(ml_dtypes.bfloat16)
            xt[s, D, :] = 1.0
            o = s * 4 * D
            wt[:D, o : o + D] = Wq[h] * scale
            wt[D, o : o + D] = bq[h] * scale
            wt[:D, o + D : o + 2 * D] = Wk[h]
            wt[D, o + D : o + 2 * D] = bk[h]
            wt[:D, o + 2 * D : o + 3 * D] = Wv[h]
            wt[D, o + 2 * D : o + 3 * D] = bv[h]
            wt[:D, o + 3 * D : o + 4 * D] = Wo[h]
            wt[D, o + 3 * D : o + 4 * D] = bo[h]
        in_maps.append({"xt": xt, "wt": wt.astype(ml_dtypes.bfloat16)})
    return in_maps


def _gather(results):
    out = np.empty((B, N, H, D), np.float32)
    for c in range(NCORES):
        ot = results[c]["ot"]
        for s in range(PPC):
            b, h = divmod(c * PPC + s, H)
            out[b, :, h, :] = ot[s].T
    return out


def run(in_maps, loop_n=1, **kw):
    nc = _build(loop_n)
    return bass_utils.run_bass_kernel_spmd(
        nc, in_maps, core_ids=list(range(NCORES)), **kw
    )


def kernel(x, Wq, bq, Wk, bk, Wv, bv, Wo, bo):
    in_maps = _host_prep(x, Wq, bq, Wk, bk, Wv, bv, Wo, bo)
    res = run(in_maps)
    return _gather(res.results)
